# revision 3
# baseline (speedup 1.0000x reference)
"""APNB (asymmetric pyramid non-local block) on 8 TRN2 NeuronCores.

Data-parallel: one batch sample per core. Per core (x: [512, 9216] of one
sample):

  Algorithmic restructure: psp_pool(conv1x1(x, W, b)) == W @ psp_pool(x) + b
  (both linear), so the k/v convolutions over the full 96x96 image collapse
  to tiny matmuls on the 110 pooled vectors.

  Pass 1 (streams x + xT from HBM, bf16):
    - q = Wq @ x + bq                  (PE, per chunk; kept resident bf16)
    - pooledT = Mpool.T @ xT           (PE, accumulating in one PSUM bank)
  Finalize:
    - pooled = pooledT.T               (PE transpose)
    - k_pool = Wk @ pooled + bk        (PE + ACT bias)
    - v_poolT = pooled.T @ Wv.T + bv   (PE, rank-1 bias matmul)
  Pass 2 (per 512-column chunk; streams out to HBM):
    - attnT  = k_pool.T @ q_chunk      [110, 512]  (PE)
    - exp    = exp(attnT)              (ACT, psum->sbuf bf16)
    - denom  = ones @ exp              (PE, replicated row sums)
    - attn   = exp * 1/denom           (DVE)
    - out    = v_poolT.T @ attn + I @ x_chunk   (PE, residual via identity
               matmul accumulated into the same PSUM bank)
    - copy psum -> sbuf (DVE/ACT) -> DMA out (fp32)

Softmax needs no max-subtraction: logits are in [-8, 8] for this problem
family (checked against the reference; exp stays finite in fp32).
"""

import numpy as np
import ml_dtypes

import concourse.bass as bass
import concourse.bacc as bacc
import concourse.tile as tile
import concourse.mybir as mybir
from concourse.bass_utils import run_bass_kernel_spmd

BF16 = ml_dtypes.bfloat16
AF = mybir.ActivationFunctionType

B = 8
C = 512          # input/value channels
O = 256          # q/k channels
H = 96
W = 96
N = H * W        # 9216
S = 110          # pooled length 1+9+36+64
PSP = (1, 3, 6, 8)
NCORES = 8
CHUNK = 1024     # columns per input DMA chunk
NBIG = N // CHUNK
SUB = 512        # columns per compute sub-chunk
KT = C // 128    # 4 contraction tiles over channels
MT_O = O // 128  # 2 output tiles for q/k
NT = N // 128    # 72 position tiles


def _build_pool_matrix() -> np.ndarray:
    m = np.zeros((N, S), dtype=np.float32)
    col = 0
    for s in PSP:
        hb, wb = H // s, W // s
        scale = 1.0 / (hb * wb)
        for i in range(s):
            for j in range(s):
                blk = np.zeros((H, W), np.float32)
                blk[i * hb:(i + 1) * hb, j * wb:(j + 1) * wb] = scale
                m[:, col] = blk.reshape(-1)
                col += 1
    assert col == S
    return m


def build_nc() -> bacc.Bacc:
    nc = bacc.Bacc("TRN2", target_bir_lowering=False, debug=False,
                   num_devices=NCORES)
    bf = mybir.dt.bfloat16
    f32 = mybir.dt.float32

    def din(name, shape, dt):
        return nc.dram_tensor(name, shape, dt, kind="ExternalInput").ap()

    x_d = din("x_bf", [C, N], bf)
    xt_d = din("xT_bf", [N, C], bf)
    mp_d = din("mpool", [N, S], bf)
    wq_d = din("wqT", [C, O], bf)
    wk_d = din("wkT", [C, O], bf)
    wv_d = din("wvT", [C, C], bf)
    bq_d = din("bq", [O, 1], f32)
    bk_d = din("bk", [O, 1], f32)
    bv_d = din("bv_bf", [1, C], bf)
    ones_d = din("ones_bf", [128, 128], bf)
    id_d = din("ident_bf", [128, 128], bf)
    out_d = nc.dram_tensor("out", [C, N], f32, kind="ExternalOutput").ap()

    xv = x_d.rearrange("(g p) n -> p g n", p=128)      # [128, 4, N]
    xtv = xt_d.rearrange("(t p) c -> p t c", p=128)    # [128, 72, C]
    mpv = mp_d.rearrange("(t p) s -> p t s", p=128)    # [128, 72, S]
    wqv = wq_d.rearrange("(k p) m -> p k m", p=128)    # [128, 4, O]
    wkv = wk_d.rearrange("(k p) m -> p k m", p=128)
    wvv = wv_d.rearrange("(k p) m -> p k m", p=128)    # [128, 4, C]
    bqv = bq_d.rearrange("(g p) o -> p g o", p=128)    # [128, 2, 1]
    bkv = bk_d.rearrange("(g p) o -> p g o", p=128)
    outv = out_d.rearrange("(g p) n -> p g n", p=128)  # [128, 4, N]

    from contextlib import ExitStack
    with tile.TileContext(nc) as tc, ExitStack() as ctx:
        consts = ctx.enter_context(tc.tile_pool(name="consts", bufs=1))
        resid = ctx.enter_context(tc.tile_pool(name="resid", bufs=1))

        wq_sb = consts.tile([128, KT, O], bf)
        nc.sync.dma_start(out=wq_sb, in_=wqv)
        wk_sb = consts.tile([128, KT, O], bf)
        nc.sync.dma_start(out=wk_sb, in_=wkv)
        wv_sb = consts.tile([128, KT, C], bf)
        nc.sync.dma_start(out=wv_sb, in_=wvv)
        mp_sb = consts.tile([128, NT, S], bf)
        nc.sync.dma_start(out=mp_sb, in_=mpv)
        bq_sb = consts.tile([128, MT_O, 1], f32)
        nc.sync.dma_start(out=bq_sb, in_=bqv)
        bk_sb = consts.tile([128, MT_O, 1], f32)
        nc.sync.dma_start(out=bk_sb, in_=bkv)
        bv_sb = consts.tile([1, C], bf)
        nc.sync.dma_start(out=bv_sb, in_=bv_d)
        ones_sb = consts.tile([128, 128], bf)
        nc.sync.dma_start(out=ones_sb, in_=ones_d)
        id_sb = consts.tile([128, 128], bf)
        nc.sync.dma_start(out=id_sb, in_=id_d)

        x_sb = resid.tile([128, KT, N], bf)       # resident input, bf16
        q_sb = resid.tile([128, MT_O, N], bf)     # resident query, bf16

        k_pool_sb = consts.tile([128, MT_O, S], bf)
        vT_sb = consts.tile([110, C], bf)
        pooledT_sb = consts.tile([110, C], bf)
        pooled_sb = consts.tile([128, KT, S], bf)

        # ---------------- pass 1: stream x, q conv + pooling ----------------
        with tc.tile_pool(name="p1ps", bufs=3, space="PSUM") as p1ps, \
             tc.tile_pool(name="poolps", bufs=1, space="PSUM") as poolps, \
             tc.tile_pool(name="xtp", bufs=2) as xtp:
            pooledT_ps = poolps.tile([110, C], mybir.dt.float32)
            for ci in range(NBIG):
                nsl = slice(ci * CHUNK, (ci + 1) * CHUNK)
                nc.sync.dma_start(out=x_sb[:, :, nsl], in_=xv[:, :, nsl])
                xt_t = xtp.tile([128, CHUNK // 128, C], bf, tag="xt")
                nc.sync.dma_start(
                    out=xt_t, in_=xtv[:, ci * (CHUNK // 128):(ci + 1) * (CHUNK // 128), :])
                for sub in range(CHUNK // SUB):
                    ns2 = slice(ci * CHUNK + sub * SUB, ci * CHUNK + (sub + 1) * SUB)
                    for m in range(MT_O):
                        q_ps = p1ps.tile([128, SUB], mybir.dt.float32, tag="qps")
                        for k in range(KT):
                            nc.tensor.matmul(
                                q_ps, wq_sb[:, k, m * 128:(m + 1) * 128],
                                x_sb[:, k, ns2], start=(k == 0), stop=(k == KT - 1))
                        nc.scalar.activation(
                            q_sb[:, m, ns2], q_ps, AF.Identity,
                            bias=bq_sb[:, m, :], scale=1.0)
                for kt in range(CHUNK // 128):
                    kg = ci * (CHUNK // 128) + kt
                    nc.tensor.matmul(
                        pooledT_ps, mp_sb[:, kg, :], xt_t[:, kt, :],
                        start=(kg == 0), stop=(kg == NT - 1),
                        skip_group_check=True)

            # ---------------- finalize pools ----------------
            nc.scalar.copy(pooledT_sb, pooledT_ps)
            for c in range(KT):
                tr_ps = p1ps.tile([128, S], mybir.dt.bfloat16, tag="trps")
                nc.tensor.transpose(
                    tr_ps, pooledT_sb[:, c * 128:(c + 1) * 128],
                    id_sb[:110, :110])
                nc.scalar.copy(pooled_sb[:, c, :], tr_ps)
            for m in range(MT_O):
                kp_ps = p1ps.tile([128, S], mybir.dt.float32, tag="trps")
                for k in range(KT):
                    nc.tensor.matmul(
                        kp_ps, wk_sb[:, k, m * 128:(m + 1) * 128],
                        pooled_sb[:, k, :], start=(k == 0), stop=(k == KT - 1))
                nc.scalar.activation(
                    k_pool_sb[:, m, :], kp_ps, AF.Identity,
                    bias=bk_sb[:, m, :], scale=1.0)
            vp_ps = p1ps.tile([110, C], mybir.dt.float32, tag="qps")
            for k in range(KT):
                nc.tensor.matmul(vp_ps, pooled_sb[:, k, :], wv_sb[:, k, :],
                                 start=(k == 0), stop=False,
                                 skip_group_check=True)
            nc.tensor.matmul(vp_ps, ones_sb[0:1, :110], bv_sb,
                             start=False, stop=True, skip_group_check=True)
            nc.scalar.copy(vT_sb, vp_ps)

        # ---------------- pass 2: attention + output ----------------
        with tc.tile_pool(name="p2ps", bufs=2, space="PSUM") as p2ps, \
             tc.tile_pool(name="pops", bufs=4, space="PSUM") as pops, \
             tc.tile_pool(name="p2sb", bufs=3) as p2sb, \
             tc.tile_pool(name="outp", bufs=3) as outp:
            for ci in range(N // SUB):
                ns2 = slice(ci * SUB, (ci + 1) * SUB)
                at_ps = p2ps.tile([110, SUB], mybir.dt.float32, tag="at")
                for kt in range(MT_O):
                    nc.tensor.matmul(at_ps, k_pool_sb[:, kt, :],
                                     q_sb[:, kt, ns2],
                                     start=(kt == 0), stop=(kt == MT_O - 1))
                exp_sb = p2sb.tile([110, SUB], mybir.dt.bfloat16, tag="exp")
                nc.scalar.activation(exp_sb, at_ps, AF.Exp)
                db_ps = p2ps.tile([110, SUB], mybir.dt.float32, tag="db")
                nc.tensor.matmul(db_ps, ones_sb[:110, :110], exp_sb,
                                 start=True, stop=True)
                recip_sb = p2sb.tile([110, SUB], mybir.dt.bfloat16, tag="recip")
                with nc.allow_low_precision("softmax weights tolerate bf16"):
                    nc.vector.reciprocal(recip_sb, db_ps)
                attn_sb = p2sb.tile([110, SUB], mybir.dt.bfloat16, tag="attn")
                nc.vector.tensor_mul(attn_sb, exp_sb, recip_sb)
                out_t = outp.tile([128, KT, SUB], mybir.dt.float32, tag="out")
                for c in range(KT):
                    o_ps = pops.tile([128, SUB], mybir.dt.float32, tag="ops")
                    nc.tensor.matmul(o_ps, vT_sb[:, c * 128:(c + 1) * 128],
                                     attn_sb, start=True, stop=False,
                                     skip_group_check=True)
                    nc.tensor.matmul(o_ps, id_sb, x_sb[:, c, ns2],
                                     start=False, stop=True,
                                     skip_group_check=True)
                    if c < 2:
                        nc.vector.tensor_copy(out_t[:, c, :], o_ps)
                    else:
                        nc.scalar.copy(out_t[:, c, :], o_ps)
                nc.sync.dma_start(out=outv[:, :, ns2], in_=out_t)

    nc.compile()
    return nc


_NC_CACHE = None


def _get_nc() -> bacc.Bacc:
    global _NC_CACHE
    if _NC_CACHE is None:
        _NC_CACHE = build_nc()
    return _NC_CACHE


def _prep_in_maps(x, Wq, bq, Wk, bk, Wv, bv):
    shared = {
        "mpool": _build_pool_matrix().astype(BF16),
        "wqT": np.ascontiguousarray(Wq.T).astype(BF16),
        "wkT": np.ascontiguousarray(Wk.T).astype(BF16),
        "wvT": np.ascontiguousarray(Wv.T).astype(BF16),
        "bq": np.ascontiguousarray(bq.reshape(O, 1)).astype(np.float32),
        "bk": np.ascontiguousarray(bk.reshape(O, 1)).astype(np.float32),
        "bv_bf": np.ascontiguousarray(bv.reshape(1, C)).astype(BF16),
        "ones_bf": np.ones((128, 128), dtype=BF16),
        "ident_bf": np.eye(128, dtype=np.float32).astype(BF16),
    }
    in_maps = []
    for i in range(NCORES):
        xi = np.ascontiguousarray(x[i].reshape(C, N))
        m = dict(shared)
        m["x_bf"] = xi.astype(BF16)
        m["xT_bf"] = np.ascontiguousarray(xi.T).astype(BF16)
        in_maps.append(m)
    return in_maps


def _install_ntff_hook():
    """The agent image ships no antenv.axon_hooks module, so trace=True
    under axon crashes on import. Recreate the tiny hook-holder module and
    register trn_boot's ctypes NTFF hook so neuron-profile timing works."""
    import sys
    import types
    if "antenv.axon_hooks" in sys.modules:
        return
    mod = types.ModuleType("antenv.axon_hooks")
    holder = {"h": None}
    mod.set_axon_ntff_profile_hook = lambda h: holder.__setitem__("h", h)
    mod.get_axon_ntff_profile_hook = lambda: holder["h"]
    sys.modules["antenv.axon_hooks"] = mod
    try:
        import antenv
        antenv.axon_hooks = mod
    except ImportError:
        pass
    try:
        from trn_agent_boot.trn_boot import _ntff_profile_via_ctypes
        mod.set_axon_ntff_profile_hook(
            _ntff_profile_via_ctypes("/opt/axon/libaxon_pjrt.so"))
    except Exception as e:  # degrade to no profiling
        print(f"ntff hook install failed: {e}")


def _run(trace: bool, **inputs):
    if trace:
        _install_ntff_hook()
        import concourse.bass_utils as bu
        bu.upload_artifacts = lambda tmpdir: tmpdir  # no cloud bucket here
    nc = _get_nc()
    in_maps = _prep_in_maps(
        inputs["x"], inputs["Wq"], inputs["bq"], inputs["Wk"], inputs["bk"],
        inputs["Wv"], inputs["bv"])
    res = run_bass_kernel_spmd(nc, in_maps, core_ids=list(range(NCORES)),
                               trace=trace)
    out = np.stack([
        np.asarray(res.results[i]["out"]).reshape(C, H, W)
        for i in range(NCORES)
    ]).astype(np.float32)
    return out, res


def kernel(**inputs) -> np.ndarray:
    out, _ = _run(False, **inputs)
    return out


def kernel_profiled(**inputs):
    out, res = _run(True, **inputs)
    return out, res


# revision 6
# speedup vs baseline: 1.2327x; 1.2327x over previous
"""APNB (asymmetric pyramid non-local block) on 8 TRN2 NeuronCores.

Data-parallel: one batch sample per core. Per core (x: [512, 9216] of one
sample):

  Algorithmic restructure: psp_pool(conv1x1(x, W, b)) == W @ psp_pool(x) + b
  (both linear), so the k/v convolutions over the full 96x96 image collapse
  to tiny matmuls on the 110 pooled vectors.

  Pass 1 (streams x + xT from HBM, bf16):
    - q = Wq @ x + bq                  (PE, per chunk; kept resident bf16)
    - pooledT = Mpool.T @ xT           (PE, accumulating in one PSUM bank)
  Finalize:
    - pooled = pooledT.T               (PE transpose)
    - k_pool = Wk @ pooled + bk        (PE + ACT bias)
    - v_poolT = pooled.T @ Wv.T + bv   (PE, rank-1 bias matmul)
  Pass 2 (per 512-column chunk; streams out to HBM):
    - attnT  = k_pool.T @ q_chunk      [110, 512]  (PE)
    - exp    = exp(attnT)              (ACT, psum->sbuf bf16)
    - denom  = ones @ exp              (PE, replicated row sums)
    - attn   = exp * 1/denom           (DVE)
    - out    = v_poolT.T @ attn + I @ x_chunk   (PE, residual via identity
               matmul accumulated into the same PSUM bank)
    - copy psum -> sbuf (DVE/ACT) -> DMA out (fp32)

Softmax needs no max-subtraction: logits are in [-8, 8] for this problem
family (checked against the reference; exp stays finite in fp32).
"""

import numpy as np
import ml_dtypes

import concourse.bass as bass
import concourse.bacc as bacc
import concourse.tile as tile
import concourse.mybir as mybir
from concourse.bass_utils import run_bass_kernel_spmd

BF16 = ml_dtypes.bfloat16
AF = mybir.ActivationFunctionType

B = 8
C = 512          # input/value channels
O = 256          # q/k channels
H = 96
W = 96
N = H * W        # 9216
S = 110          # pooled length 1+9+36+64
PSP = (1, 3, 6, 8)
NCORES = 8
CHUNK = 1024     # columns per input DMA chunk
NBIG = N // CHUNK
SUB = 512        # columns per compute sub-chunk
KT = C // 128    # 4 contraction tiles over channels
MT_O = O // 128  # 2 output tiles for q/k
NT = N // 128    # 72 position tiles


def _build_pool_matrix() -> np.ndarray:
    m = np.zeros((N, S), dtype=np.float32)
    col = 0
    for s in PSP:
        hb, wb = H // s, W // s
        scale = 1.0 / (hb * wb)
        for i in range(s):
            for j in range(s):
                blk = np.zeros((H, W), np.float32)
                blk[i * hb:(i + 1) * hb, j * wb:(j + 1) * wb] = scale
                m[:, col] = blk.reshape(-1)
                col += 1
    assert col == S
    return m


def build_nc() -> bacc.Bacc:
    nc = bacc.Bacc("TRN2", target_bir_lowering=False, debug=False,
                   num_devices=NCORES)
    bf = mybir.dt.bfloat16
    f32 = mybir.dt.float32

    def din(name, shape, dt):
        return nc.dram_tensor(name, shape, dt, kind="ExternalInput").ap()

    x_d = din("x_bf", [C, N], bf)
    xt_d = din("xT_bf", [N, C], bf)
    mp_d = din("mpool", [N, S], bf)
    wq_d = din("wqT", [C, O], bf)
    wk_d = din("wkT", [C, O], bf)
    wv_d = din("wvT", [C, C], bf)
    bq_d = din("bq", [O, 1], f32)
    bk_d = din("bk", [O, 1], f32)
    bv_d = din("bv_bf", [1, C], bf)
    ones_d = din("ones_bf", [128, 128], bf)
    id_d = din("ident_bf", [128, 128], bf)
    out_d = nc.dram_tensor("out", [C, N], f32, kind="ExternalOutput").ap()

    xv = x_d.rearrange("(g p) n -> p g n", p=128)      # [128, 4, N]
    xtv = xt_d.rearrange("(t p) c -> p t c", p=128)    # [128, 72, C]
    mpv = mp_d.rearrange("(t p) s -> p t s", p=128)    # [128, 72, S]
    wqv = wq_d.rearrange("(k p) m -> p k m", p=128)    # [128, 4, O]
    wkv = wk_d.rearrange("(k p) m -> p k m", p=128)
    wvv = wv_d.rearrange("(k p) m -> p k m", p=128)    # [128, 4, C]
    bqv = bq_d.rearrange("(g p) o -> p g o", p=128)    # [128, 2, 1]
    bkv = bk_d.rearrange("(g p) o -> p g o", p=128)
    outv = out_d.rearrange("(g p) n -> p g n", p=128)  # [128, 4, N]

    from contextlib import ExitStack
    with tile.TileContext(nc) as tc, ExitStack() as ctx:
        consts = ctx.enter_context(tc.tile_pool(name="consts", bufs=1))
        resid = ctx.enter_context(tc.tile_pool(name="resid", bufs=1))

        # consts go on the ACT HWDGE queue so they don't serialize ahead of
        # the x/xT streams on the SP queue
        wq_sb = consts.tile([128, KT, O], bf)
        nc.scalar.dma_start(out=wq_sb, in_=wqv)
        wk_sb = consts.tile([128, KT, O], bf)
        nc.scalar.dma_start(out=wk_sb, in_=wkv)
        wv_sb = consts.tile([128, KT, C], bf)
        nc.scalar.dma_start(out=wv_sb, in_=wvv)
        mp_sb = consts.tile([128, NT, S], bf)
        nc.scalar.dma_start(out=mp_sb, in_=mpv)
        bq_sb = consts.tile([128, MT_O, 1], f32)
        nc.scalar.dma_start(out=bq_sb, in_=bqv)
        bk_sb = consts.tile([128, MT_O, 1], f32)
        nc.scalar.dma_start(out=bk_sb, in_=bkv)
        bv_sb = consts.tile([1, C], bf)
        nc.scalar.dma_start(out=bv_sb, in_=bv_d)
        ones_sb = consts.tile([128, 128], bf)
        nc.scalar.dma_start(out=ones_sb, in_=ones_d)
        id_sb = consts.tile([128, 128], bf)
        nc.scalar.dma_start(out=id_sb, in_=id_d)

        x_sb = resid.tile([128, KT, N], bf)       # resident input, bf16
        q_sb = resid.tile([128, MT_O, N], bf)     # resident query, bf16

        k_pool_sb = consts.tile([128, MT_O, S], bf)
        vT_sb = consts.tile([110, C], bf)
        pooledT_sb = consts.tile([110, C], bf)
        pooled_sb = consts.tile([128, KT, S], bf)

        # ---------------- pass 1: stream x, q conv + pooling ----------------
        with tc.tile_pool(name="p1ps", bufs=3, space="PSUM") as p1ps, \
             tc.tile_pool(name="poolps", bufs=1, space="PSUM") as poolps, \
             tc.tile_pool(name="xtp", bufs=2) as xtp:
            pooledT_ps = poolps.tile([110, C], mybir.dt.float32)
            for ci in range(NBIG):
                nsl = slice(ci * CHUNK, (ci + 1) * CHUNK)
                nc.sync.dma_start(out=x_sb[:, :, nsl], in_=xv[:, :, nsl])
                xt_t = xtp.tile([128, CHUNK // 128, C], bf, tag="xt")
                nc.scalar.dma_start(
                    out=xt_t, in_=xtv[:, ci * (CHUNK // 128):(ci + 1) * (CHUNK // 128), :])
                for sub in range(CHUNK // SUB):
                    ns2 = slice(ci * CHUNK + sub * SUB, ci * CHUNK + (sub + 1) * SUB)
                    for m in range(MT_O):
                        q_ps = p1ps.tile([128, SUB], mybir.dt.float32, tag="qps")
                        for k in range(KT):
                            nc.tensor.matmul(
                                q_ps, wq_sb[:, k, m * 128:(m + 1) * 128],
                                x_sb[:, k, ns2], start=(k == 0), stop=(k == KT - 1))
                        nc.scalar.activation(
                            q_sb[:, m, ns2], q_ps, AF.Identity,
                            bias=bq_sb[:, m, :], scale=1.0)
                for kt in range(CHUNK // 128):
                    kg = ci * (CHUNK // 128) + kt
                    nc.tensor.matmul(
                        pooledT_ps, mp_sb[:, kg, :], xt_t[:, kt, :],
                        start=(kg == 0), stop=(kg == NT - 1),
                        skip_group_check=True)

            # ---------------- finalize pools ----------------
            nc.scalar.copy(pooledT_sb, pooledT_ps)
            for c in range(KT):
                tr_ps = p1ps.tile([128, S], mybir.dt.bfloat16, tag="trps")
                nc.tensor.transpose(
                    tr_ps, pooledT_sb[:, c * 128:(c + 1) * 128],
                    id_sb[:110, :110])
                nc.scalar.copy(pooled_sb[:, c, :], tr_ps)
            for m in range(MT_O):
                kp_ps = p1ps.tile([128, S], mybir.dt.float32, tag="trps")
                for k in range(KT):
                    nc.tensor.matmul(
                        kp_ps, wk_sb[:, k, m * 128:(m + 1) * 128],
                        pooled_sb[:, k, :], start=(k == 0), stop=(k == KT - 1))
                nc.scalar.activation(
                    k_pool_sb[:, m, :], kp_ps, AF.Identity,
                    bias=bk_sb[:, m, :], scale=1.0)
            vp_ps = p1ps.tile([110, C], mybir.dt.float32, tag="qps")
            for k in range(KT):
                nc.tensor.matmul(vp_ps, pooled_sb[:, k, :], wv_sb[:, k, :],
                                 start=(k == 0), stop=False,
                                 skip_group_check=True)
            nc.tensor.matmul(vp_ps, ones_sb[0:1, :110], bv_sb,
                             start=False, stop=True, skip_group_check=True)
            nc.scalar.copy(vT_sb, vp_ps)

        # ---------------- pass 2: attention + output ----------------
        with tc.tile_pool(name="p2ps", bufs=2, space="PSUM") as p2ps, \
             tc.tile_pool(name="pops", bufs=4, space="PSUM") as pops, \
             tc.tile_pool(name="p2sb", bufs=3) as p2sb, \
             tc.tile_pool(name="outp", bufs=3) as outp:
            for ci in range(N // SUB):
                ns2 = slice(ci * SUB, (ci + 1) * SUB)
                at_ps = p2ps.tile([110, SUB], mybir.dt.float32, tag="at")
                for kt in range(MT_O):
                    nc.tensor.matmul(at_ps, k_pool_sb[:, kt, :],
                                     q_sb[:, kt, ns2],
                                     start=(kt == 0), stop=(kt == MT_O - 1))
                exp_sb = p2sb.tile([110, SUB], mybir.dt.bfloat16, tag="exp")
                nc.scalar.activation(exp_sb, at_ps, AF.Exp)
                db_ps = p2ps.tile([110, SUB], mybir.dt.float32, tag="db")
                nc.tensor.matmul(db_ps, ones_sb[:110, :110], exp_sb,
                                 start=True, stop=True)
                recip_sb = p2sb.tile([110, SUB], mybir.dt.float32, tag="recip")
                nc.vector.reciprocal_approx_fast(recip_sb, db_ps)
                attn_sb = p2sb.tile([110, SUB], mybir.dt.bfloat16, tag="attn")
                with nc.allow_low_precision("softmax weights tolerate bf16"):
                    nc.gpsimd.tensor_mul(attn_sb, exp_sb, recip_sb)
                out_t = outp.tile([128, KT, SUB], mybir.dt.float32, tag="out")
                for c in range(KT):
                    o_ps = pops.tile([128, SUB], mybir.dt.float32, tag="ops")
                    if c < 2:
                        # residual add fused into the psum->sbuf move (DVE)
                        nc.tensor.matmul(o_ps, vT_sb[:, c * 128:(c + 1) * 128],
                                         attn_sb, start=True, stop=True,
                                         skip_group_check=True)
                        with nc.allow_low_precision("fp32 psum + bf16 residual"):
                            nc.vector.tensor_add(out_t[:, c, :], o_ps,
                                                 x_sb[:, c, ns2])
                    else:
                        # residual add via identity matmul (PE), copy on ACT
                        nc.tensor.matmul(o_ps, vT_sb[:, c * 128:(c + 1) * 128],
                                         attn_sb, start=True, stop=False,
                                         skip_group_check=True)
                        nc.tensor.matmul(o_ps, id_sb, x_sb[:, c, ns2],
                                         start=False, stop=True,
                                         skip_group_check=True)
                        nc.scalar.copy(out_t[:, c, :], o_ps)
                nc.sync.dma_start(out=outv[:, :, ns2], in_=out_t)

    nc.compile()
    return nc


_NC_CACHE = None


def _get_nc() -> bacc.Bacc:
    global _NC_CACHE
    if _NC_CACHE is None:
        _NC_CACHE = build_nc()
    return _NC_CACHE


def _prep_in_maps(x, Wq, bq, Wk, bk, Wv, bv):
    shared = {
        "mpool": _build_pool_matrix().astype(BF16),
        "wqT": np.ascontiguousarray(Wq.T).astype(BF16),
        "wkT": np.ascontiguousarray(Wk.T).astype(BF16),
        "wvT": np.ascontiguousarray(Wv.T).astype(BF16),
        "bq": np.ascontiguousarray(bq.reshape(O, 1)).astype(np.float32),
        "bk": np.ascontiguousarray(bk.reshape(O, 1)).astype(np.float32),
        "bv_bf": np.ascontiguousarray(bv.reshape(1, C)).astype(BF16),
        "ones_bf": np.ones((128, 128), dtype=BF16),
        "ident_bf": np.eye(128, dtype=np.float32).astype(BF16),
    }
    in_maps = []
    for i in range(NCORES):
        xi = np.ascontiguousarray(x[i].reshape(C, N))
        m = dict(shared)
        m["x_bf"] = xi.astype(BF16)
        m["xT_bf"] = np.ascontiguousarray(xi.T).astype(BF16)
        in_maps.append(m)
    return in_maps


def _install_ntff_hook():
    """The agent image ships no antenv.axon_hooks module, so trace=True
    under axon crashes on import. Recreate the tiny hook-holder module and
    register trn_boot's ctypes NTFF hook so neuron-profile timing works."""
    import sys
    import types
    if "antenv.axon_hooks" in sys.modules:
        return
    mod = types.ModuleType("antenv.axon_hooks")
    holder = {"h": None}
    mod.set_axon_ntff_profile_hook = lambda h: holder.__setitem__("h", h)
    mod.get_axon_ntff_profile_hook = lambda: holder["h"]
    sys.modules["antenv.axon_hooks"] = mod
    try:
        import antenv
        antenv.axon_hooks = mod
    except ImportError:
        pass
    try:
        from trn_agent_boot.trn_boot import _ntff_profile_via_ctypes
        mod.set_axon_ntff_profile_hook(
            _ntff_profile_via_ctypes("/opt/axon/libaxon_pjrt.so"))
    except Exception as e:  # degrade to no profiling
        print(f"ntff hook install failed: {e}")


def _run(trace: bool, **inputs):
    if trace:
        _install_ntff_hook()
        import concourse.bass_utils as bu
        bu.upload_artifacts = lambda tmpdir: tmpdir  # no cloud bucket here
    nc = _get_nc()
    in_maps = _prep_in_maps(
        inputs["x"], inputs["Wq"], inputs["bq"], inputs["Wk"], inputs["bk"],
        inputs["Wv"], inputs["bv"])
    res = run_bass_kernel_spmd(nc, in_maps, core_ids=list(range(NCORES)),
                               trace=trace)
    out = np.stack([
        np.asarray(res.results[i]["out"]).reshape(C, H, W)
        for i in range(NCORES)
    ]).astype(np.float32)
    return out, res


def kernel(**inputs) -> np.ndarray:
    out, _ = _run(False, **inputs)
    return out


def kernel_profiled(**inputs):
    out, res = _run(True, **inputs)
    return out, res


# revision 16
# speedup vs baseline: 1.2871x; 1.0442x over previous
"""APNB (asymmetric pyramid non-local block) on 8 TRN2 NeuronCores.

Data-parallel: one batch sample per core. Per core (x: [512, 9216] of one
sample):

  Algorithmic restructure: psp_pool(conv1x1(x, W, b)) == W @ psp_pool(x) + b
  (both linear), so the k/v convolutions over the full 96x96 image collapse
  to tiny matmuls on the 110 pooled vectors.

  Pass 1 (streams x + xT from HBM, bf16):
    - q = Wq @ x + bq                  (PE, per chunk; kept resident bf16)
    - pooledT = Mpool.T @ xT           (PE, accumulating in one PSUM bank)
  Finalize:
    - pooled = pooledT.T               (PE transpose)
    - k_pool = Wk @ pooled + bk        (PE + ACT bias)
    - v_poolT = pooled.T @ Wv.T + bv   (PE, rank-1 bias matmul)
  Pass 2 (per 512-column chunk; streams out to HBM):
    - attnT  = k_pool.T @ q_chunk      [110, 512]  (PE)
    - exp    = exp(attnT)              (ACT, psum->sbuf bf16)
    - denom  = ones @ exp              (PE, replicated row sums)
    - attn   = exp * 1/denom           (DVE)
    - out    = v_poolT.T @ attn + I @ x_chunk   (PE, residual via identity
               matmul accumulated into the same PSUM bank)
    - copy psum -> sbuf (DVE/ACT) -> DMA out (fp32)

Softmax needs no max-subtraction: logits are in [-8, 8] for this problem
family (checked against the reference; exp stays finite in fp32).
"""

import numpy as np
import ml_dtypes

import concourse.bass as bass
import concourse.bacc as bacc
import concourse.tile as tile
import concourse.mybir as mybir
from concourse.bass_utils import run_bass_kernel_spmd

BF16 = ml_dtypes.bfloat16
AF = mybir.ActivationFunctionType

B = 8
C = 512          # input/value channels
O = 256          # q/k channels
H = 96
W = 96
N = H * W        # 9216
S = 110          # pooled length 1+9+36+64
PSP = (1, 3, 6, 8)
NCORES = 8
CHUNK = 1024     # columns per input DMA chunk
NBIG = N // CHUNK
SUB = 512        # columns per compute sub-chunk
KT = C // 128    # 4 contraction tiles over channels
MT_O = O // 128  # 2 output tiles for q/k
NT = N // 128    # 72 position tiles


def _build_pool_matrix() -> np.ndarray:
    m = np.zeros((N, S), dtype=np.float32)
    col = 0
    for s in PSP:
        hb, wb = H // s, W // s
        scale = 1.0 / (hb * wb)
        for i in range(s):
            for j in range(s):
                blk = np.zeros((H, W), np.float32)
                blk[i * hb:(i + 1) * hb, j * wb:(j + 1) * wb] = scale
                m[:, col] = blk.reshape(-1)
                col += 1
    assert col == S
    return m


def build_nc() -> bacc.Bacc:
    nc = bacc.Bacc("TRN2", target_bir_lowering=False, debug=False,
                   num_devices=NCORES)
    bf = mybir.dt.bfloat16
    f32 = mybir.dt.float32

    def din(name, shape, dt):
        return nc.dram_tensor(name, shape, dt, kind="ExternalInput").ap()

    x_d = din("x_bf", [C, N], bf)
    xt_d = din("xT_bf", [N, C], bf)
    mp_d = din("mpool", [N, S], bf)
    wq_d = din("wqT", [C, O], bf)
    wk_d = din("wkT", [C, O], bf)
    wv_d = din("wvT", [C, C], bf)
    bq_d = din("bq_bf", [1, O], bf)
    bk_d = din("bk_bf", [1, O], bf)
    bv_d = din("bv_bf", [1, C], bf)
    ones_d = din("ones_bf", [128, 512], bf)
    id_d = din("ident_bf", [128, 128], bf)
    out_d = nc.dram_tensor("out", [C, N], f32, kind="ExternalOutput").ap()

    xv = x_d.rearrange("(g p) n -> p g n", p=128)      # [128, 4, N]
    xtv = xt_d.rearrange("(t p) c -> p t c", p=128)    # [128, 72, C]
    mpv = mp_d.rearrange("(t p) s -> p t s", p=128)    # [128, 72, S]
    wqv = wq_d.rearrange("(k p) m -> p k m", p=128)    # [128, 4, O]
    wkv = wk_d.rearrange("(k p) m -> p k m", p=128)
    wvv = wv_d.rearrange("(k p) m -> p k m", p=128)    # [128, 4, C]
    outv = out_d.rearrange("(g p) n -> p g n", p=128)  # [128, 4, N]

    from contextlib import ExitStack
    with tile.TileContext(nc) as tc, ExitStack() as ctx:
        consts = ctx.enter_context(tc.tile_pool(name="consts", bufs=1))
        resid = ctx.enter_context(tc.tile_pool(name="resid", bufs=1))

        # consts + xT go through gpsimd SWDGE so they never contend with the
        # x/out streams on the SP HWDGE queue, nor with ACT compute
        wq_sb = consts.tile([128, KT, O], bf)
        nc.gpsimd.dma_start(out=wq_sb, in_=wqv)
        bq_sb = consts.tile([1, O], bf)
        nc.gpsimd.dma_start(out=bq_sb, in_=bq_d)
        ones_sb = consts.tile([128, 512], bf)
        nc.gpsimd.dma_start(out=ones_sb, in_=ones_d)
        mp_sb = consts.tile([128, NT, S], bf)
        nc.gpsimd.dma_start(out=mp_sb, in_=mpv)
        wk_sb = consts.tile([128, KT, O], bf)
        nc.gpsimd.dma_start(out=wk_sb, in_=wkv)
        wv_sb = consts.tile([128, KT, C], bf)
        nc.gpsimd.dma_start(out=wv_sb, in_=wvv)
        bk_sb = consts.tile([1, O], bf)
        nc.gpsimd.dma_start(out=bk_sb, in_=bk_d)
        bv_sb = consts.tile([1, C], bf)
        nc.gpsimd.dma_start(out=bv_sb, in_=bv_d)
        id_sb = consts.tile([128, 128], bf)
        nc.gpsimd.dma_start(out=id_sb, in_=id_d)

        x_sb = resid.tile([128, KT, N], bf)       # resident input, bf16
        q_sb = resid.tile([128, MT_O, N], bf)     # resident query, bf16

        k_pool_sb = consts.tile([128, MT_O, S], bf)
        vT_sb = consts.tile([110, C], bf)
        pooledT_sb = consts.tile([110, C], bf)
        pooled_sb = consts.tile([128, KT, S], bf)

        # ---------------- pass 1: stream x, q conv + pooling ----------------
        with tc.tile_pool(name="p1ps", bufs=2, space="PSUM") as p1ps, \
             tc.tile_pool(name="poolps", bufs=1, space="PSUM") as poolps, \
             tc.tile_pool(name="xtp", bufs=2) as xtp:
            pooledT_ps = poolps.tile([110, C], mybir.dt.float32)
            for ci in range(NBIG):
                nsl = slice(ci * CHUNK, (ci + 1) * CHUNK)
                nc.sync.dma_start(out=x_sb[:, :, nsl], in_=xv[:, :, nsl])
                xt_t = xtp.tile([128, CHUNK // 128, C], bf, tag="xt")
                nc.gpsimd.dma_start(
                    out=xt_t, in_=xtv[:, ci * (CHUNK // 128):(ci + 1) * (CHUNK // 128), :])
                for sub in range(CHUNK // SUB):
                    ns2 = slice(ci * CHUNK + sub * SUB, ci * CHUNK + (sub + 1) * SUB)
                    q_ps = p1ps.tile([128, MT_O, SUB], mybir.dt.float32, tag="qps")
                    for m in range(MT_O):
                        for k in range(KT):
                            nc.tensor.matmul(
                                q_ps[:, m, :], wq_sb[:, k, m * 128:(m + 1) * 128],
                                x_sb[:, k, ns2], start=(k == 0), stop=False,
                                skip_group_check=True)
                        # channel bias as a rank-1 matmul: bq_m x ones_n
                        nc.tensor.matmul(
                            q_ps[:, m, :], bq_sb[0:1, m * 128:(m + 1) * 128],
                            ones_sb[0:1, 0:SUB], start=False, stop=True,
                            skip_group_check=True)
                    nc.scalar.copy(q_sb[:, :, ns2], q_ps)
                for kt in range(CHUNK // 128):
                    kg = ci * (CHUNK // 128) + kt
                    nc.tensor.matmul(
                        pooledT_ps, mp_sb[:, kg, :], xt_t[:, kt, :],
                        start=(kg == 0), stop=(kg == NT - 1),
                        skip_group_check=True)

            # ---------------- finalize pools ----------------
            nc.scalar.copy(pooledT_sb, pooledT_ps)
            for c in range(KT):
                tr_ps = p1ps.tile([128, S], mybir.dt.bfloat16, tag="trps")
                nc.tensor.transpose(
                    tr_ps, pooledT_sb[:, c * 128:(c + 1) * 128],
                    id_sb[:110, :110])
                nc.scalar.copy(pooled_sb[:, c, :], tr_ps)
            kp_ps = p1ps.tile([128, MT_O, 128], mybir.dt.float32, tag="trps")
            for m in range(MT_O):
                for k in range(KT):
                    nc.tensor.matmul(
                        kp_ps[:, m, 0:S], wk_sb[:, k, m * 128:(m + 1) * 128],
                        pooled_sb[:, k, :], start=(k == 0), stop=False,
                        skip_group_check=True)
                nc.tensor.matmul(
                    kp_ps[:, m, 0:S], bk_sb[0:1, m * 128:(m + 1) * 128],
                    ones_sb[0:1, 0:S], start=False, stop=True,
                    skip_group_check=True)
            nc.scalar.copy(k_pool_sb, kp_ps[:, :, 0:S])
            vp_ps = p1ps.tile([110, C], mybir.dt.float32, tag="qps")
            for k in range(KT):
                nc.tensor.matmul(vp_ps, pooled_sb[:, k, :], wv_sb[:, k, :],
                                 start=(k == 0), stop=False,
                                 skip_group_check=True)
            nc.tensor.matmul(vp_ps, ones_sb[0:1, :110], bv_sb,
                             start=False, stop=True, skip_group_check=True)
            nc.scalar.copy(vT_sb, vp_ps)

        # ---------------- pass 2: attention + output ----------------
        with tc.tile_pool(name="p2ps", bufs=2, space="PSUM") as p2ps, \
             tc.tile_pool(name="pops", bufs=2, space="PSUM") as pops, \
             tc.tile_pool(name="p2sb", bufs=3) as p2sb, \
             tc.tile_pool(name="outp", bufs=3) as outp:
            for ci in range(N // SUB):
                ns2 = slice(ci * SUB, (ci + 1) * SUB)
                at_ps = p2ps.tile([110, SUB], mybir.dt.float32, tag="at")
                for kt in range(MT_O):
                    nc.tensor.matmul(at_ps, k_pool_sb[:, kt, :],
                                     q_sb[:, kt, ns2],
                                     start=(kt == 0), stop=(kt == MT_O - 1))
                exp_sb = p2sb.tile([110, SUB], mybir.dt.bfloat16, tag="exp")
                nc.scalar.activation(exp_sb, at_ps, AF.Exp)
                db_ps = p2ps.tile([110, SUB], mybir.dt.float32, tag="db")
                nc.tensor.matmul(db_ps, ones_sb[:110, :110], exp_sb,
                                 start=True, stop=True)
                recip_sb = p2sb.tile([110, SUB], mybir.dt.float32, tag="recip")
                nc.vector.reciprocal_approx_fast(recip_sb, db_ps)
                attn_sb = p2sb.tile([110, SUB], mybir.dt.bfloat16, tag="attn")
                with nc.allow_low_precision("softmax weights tolerate bf16"):
                    nc.gpsimd.tensor_mul(attn_sb, exp_sb, recip_sb)
                out_t = outp.tile([128, KT, SUB], mybir.dt.float32, tag="out")
                # c-tiles 0,1: residual add fused into the psum->sbuf move (DVE)
                o_psa = pops.tile([128, 2, SUB], mybir.dt.float32, tag="ops")
                for c in range(2):
                    nc.tensor.matmul(o_psa[:, c, :],
                                     vT_sb[:, c * 128:(c + 1) * 128],
                                     attn_sb, start=True, stop=True,
                                     skip_group_check=True)
                with nc.allow_low_precision("fp32 psum + bf16 residual"):
                    nc.vector.tensor_add(out_t[:, 0:2, :], o_psa,
                                         x_sb[:, 0:2, ns2])
                # c-tiles 2,3: residual via identity matmul (PE), copy on ACT
                o_psb = pops.tile([128, 2, SUB], mybir.dt.float32, tag="ops")
                for c in range(2, KT):
                    nc.tensor.matmul(o_psb[:, c - 2, :],
                                     vT_sb[:, c * 128:(c + 1) * 128],
                                     attn_sb, start=True, stop=False,
                                     skip_group_check=True)
                    nc.tensor.matmul(o_psb[:, c - 2, :], id_sb,
                                     x_sb[:, c, ns2],
                                     start=False, stop=True,
                                     skip_group_check=True)
                nc.scalar.copy(out_t[:, 2:4, :], o_psb)
                nc.sync.dma_start(out=outv[:, :, ns2], in_=out_t)

    nc.compile()
    return nc


_NC_CACHE = None


def _get_nc() -> bacc.Bacc:
    global _NC_CACHE
    if _NC_CACHE is None:
        _NC_CACHE = build_nc()
    return _NC_CACHE


def _prep_in_maps(x, Wq, bq, Wk, bk, Wv, bv):
    shared = {
        "mpool": _build_pool_matrix().astype(BF16),
        "wqT": np.ascontiguousarray(Wq.T).astype(BF16),
        "wkT": np.ascontiguousarray(Wk.T).astype(BF16),
        "wvT": np.ascontiguousarray(Wv.T).astype(BF16),
        "bq_bf": np.ascontiguousarray(bq.reshape(1, O)).astype(BF16),
        "bk_bf": np.ascontiguousarray(bk.reshape(1, O)).astype(BF16),
        "bv_bf": np.ascontiguousarray(bv.reshape(1, C)).astype(BF16),
        "ones_bf": np.ones((128, 512), dtype=BF16),
        "ident_bf": np.eye(128, dtype=np.float32).astype(BF16),
    }
    in_maps = []
    for i in range(NCORES):
        xi = np.ascontiguousarray(x[i].reshape(C, N))
        m = dict(shared)
        m["x_bf"] = xi.astype(BF16)
        m["xT_bf"] = np.ascontiguousarray(xi.T).astype(BF16)
        in_maps.append(m)
    return in_maps


def _install_ntff_hook():
    """The agent image ships no antenv.axon_hooks module, so trace=True
    under axon crashes on import. Recreate the tiny hook-holder module and
    register trn_boot's ctypes NTFF hook so neuron-profile timing works."""
    import sys
    import types
    if "antenv.axon_hooks" in sys.modules:
        return
    mod = types.ModuleType("antenv.axon_hooks")
    holder = {"h": None}
    mod.set_axon_ntff_profile_hook = lambda h: holder.__setitem__("h", h)
    mod.get_axon_ntff_profile_hook = lambda: holder["h"]
    sys.modules["antenv.axon_hooks"] = mod
    try:
        import antenv
        antenv.axon_hooks = mod
    except ImportError:
        pass
    try:
        from trn_agent_boot.trn_boot import _ntff_profile_via_ctypes
        mod.set_axon_ntff_profile_hook(
            _ntff_profile_via_ctypes("/opt/axon/libaxon_pjrt.so"))
    except Exception as e:  # degrade to no profiling
        print(f"ntff hook install failed: {e}")


def _run(trace: bool, **inputs):
    if trace:
        _install_ntff_hook()
        import concourse.bass_utils as bu
        bu.upload_artifacts = lambda tmpdir: tmpdir  # no cloud bucket here
    nc = _get_nc()
    in_maps = _prep_in_maps(
        inputs["x"], inputs["Wq"], inputs["bq"], inputs["Wk"], inputs["bk"],
        inputs["Wv"], inputs["bv"])
    res = run_bass_kernel_spmd(nc, in_maps, core_ids=list(range(NCORES)),
                               trace=trace)
    out = np.stack([
        np.asarray(res.results[i]["out"]).reshape(C, H, W)
        for i in range(NCORES)
    ]).astype(np.float32)
    return out, res


def kernel(**inputs) -> np.ndarray:
    out, _ = _run(False, **inputs)
    return out


def kernel_profiled(**inputs):
    out, res = _run(True, **inputs)
    return out, res


# revision 24
# speedup vs baseline: 1.3003x; 1.0102x over previous
"""APNB (asymmetric pyramid non-local block) on 8 TRN2 NeuronCores.

Data-parallel: one batch sample per core. Per core (x: [512, 9216] of one
sample):

  Algorithmic restructure: psp_pool(conv1x1(x, W, b)) == W @ psp_pool(x) + b
  (both linear), so the k/v convolutions over the full 96x96 image collapse
  to tiny matmuls on the 110 pooled vectors.

  Pass 1 (streams x + xT from HBM, bf16):
    - q = Wq @ x + bq                  (PE, per chunk; kept resident bf16)
    - pooledT = Mpool.T @ xT           (PE, accumulating in one PSUM bank)
  Finalize:
    - pooled = pooledT.T               (PE transpose)
    - k_pool = Wk @ pooled + bk        (PE + ACT bias)
    - v_poolT = pooled.T @ Wv.T + bv   (PE, rank-1 bias matmul)
  Pass 2 (per 512-column chunk; streams out to HBM):
    - attnT  = k_pool.T @ q_chunk      [110, 512]  (PE)
    - exp    = exp(attnT)              (ACT, psum->sbuf bf16)
    - denom  = ones @ exp              (PE, replicated row sums)
    - attn   = exp * 1/denom           (DVE)
    - out    = v_poolT.T @ attn + I @ x_chunk   (PE, residual via identity
               matmul accumulated into the same PSUM bank)
    - copy psum -> sbuf (DVE/ACT) -> DMA out (fp32)

Softmax needs no max-subtraction: logits are in [-8, 8] for this problem
family (checked against the reference; exp stays finite in fp32).
"""

import numpy as np
import ml_dtypes

import concourse.bass as bass
import concourse.bacc as bacc
import concourse.tile as tile
import concourse.mybir as mybir
from concourse.bass_utils import run_bass_kernel_spmd

BF16 = ml_dtypes.bfloat16
AF = mybir.ActivationFunctionType

B = 8
C = 512          # input/value channels
O = 256          # q/k channels
H = 96
W = 96
N = H * W        # 9216
S = 110          # pooled length 1+9+36+64
PSP = (1, 3, 6, 8)
NCORES = 8
CHUNK = 1024     # columns per input DMA chunk
NBIG = N // CHUNK
SUB = 512        # columns per compute sub-chunk
KT = C // 128    # 4 contraction tiles over channels
MT_O = O // 128  # 2 output tiles for q/k
NT = N // 128    # 72 position tiles


def _build_pool_matrix() -> np.ndarray:
    m = np.zeros((N, S), dtype=np.float32)
    col = 0
    for s in PSP:
        hb, wb = H // s, W // s
        scale = 1.0 / (hb * wb)
        for i in range(s):
            for j in range(s):
                blk = np.zeros((H, W), np.float32)
                blk[i * hb:(i + 1) * hb, j * wb:(j + 1) * wb] = scale
                m[:, col] = blk.reshape(-1)
                col += 1
    assert col == S
    return m


def build_nc() -> bacc.Bacc:
    nc = bacc.Bacc("TRN2", target_bir_lowering=False, debug=False,
                   num_devices=NCORES)
    bf = mybir.dt.bfloat16
    f32 = mybir.dt.float32

    def din(name, shape, dt):
        return nc.dram_tensor(name, shape, dt, kind="ExternalInput").ap()

    x_d = din("x_bf", [C, N], bf)
    xt_d = din("xT_bf", [N, C], bf)
    mp_d = din("mpool", [N, S], bf)
    wq_d = din("wqT", [C, O], bf)
    wk_d = din("wkT", [C, O], bf)
    wv_d = din("wvT", [C, C], bf)
    bq_d = din("bq", [O, 1], f32)
    bk_d = din("bk_bf", [1, O], bf)
    bv_d = din("bv_bf", [1, C], bf)
    ones_d = din("ones_bf", [128, 512], bf)
    id_d = din("ident_bf", [128, 128], bf)
    out_d = nc.dram_tensor("out", [C, N], f32, kind="ExternalOutput").ap()

    xv = x_d.rearrange("(g p) n -> p g n", p=128)      # [128, 4, N]
    xtv = xt_d.rearrange("(t p) c -> p t c", p=128)    # [128, 72, C]
    mpv = mp_d.rearrange("(t p) s -> p t s", p=128)    # [128, 72, S]
    wqv = wq_d.rearrange("(k p) m -> p k m", p=128)    # [128, 4, O]
    wkv = wk_d.rearrange("(k p) m -> p k m", p=128)
    wvv = wv_d.rearrange("(k p) m -> p k m", p=128)    # [128, 4, C]
    bqv = bq_d.rearrange("(g p) o -> p g o", p=128)    # [128, 2, 1]
    outv = out_d.rearrange("(g p) n -> p g n", p=128)  # [128, 4, N]

    from contextlib import ExitStack
    with tile.TileContext(nc) as tc, ExitStack() as ctx:
        consts = ctx.enter_context(tc.tile_pool(name="consts", bufs=1))
        resid = ctx.enter_context(tc.tile_pool(name="resid", bufs=1))

        # consts + xT go through gpsimd SWDGE so they never contend with the
        # x/out streams on the SP HWDGE queue, nor with ACT compute
        wq_sb = consts.tile([128, KT, O], bf)
        nc.gpsimd.dma_start(out=wq_sb, in_=wqv)
        bq_sb = consts.tile([128, MT_O, 1], f32)
        nc.gpsimd.dma_start(out=bq_sb, in_=bqv)
        ones_sb = consts.tile([128, 512], bf)
        nc.gpsimd.dma_start(out=ones_sb, in_=ones_d)
        mp_sb = consts.tile([128, NT, S], bf)
        nc.gpsimd.dma_start(out=mp_sb, in_=mpv)
        wk_sb = consts.tile([128, KT, O], bf)
        nc.gpsimd.dma_start(out=wk_sb, in_=wkv)
        wv_sb = consts.tile([128, KT, C], bf)
        nc.gpsimd.dma_start(out=wv_sb, in_=wvv)
        bk_sb = consts.tile([1, O], bf)
        nc.gpsimd.dma_start(out=bk_sb, in_=bk_d)
        bv_sb = consts.tile([1, C], bf)
        nc.gpsimd.dma_start(out=bv_sb, in_=bv_d)
        id_sb = consts.tile([128, 128], bf)
        nc.gpsimd.dma_start(out=id_sb, in_=id_d)

        x_sb = resid.tile([128, KT, N], bf)       # resident input, bf16
        q_sb = resid.tile([128, MT_O, N], bf)     # resident query, bf16

        k_pool_sb = consts.tile([128, MT_O, S], bf)
        vT_sb = consts.tile([110, C], bf)
        pooledT_sb = consts.tile([110, C], bf)
        pooled_sb = consts.tile([128, KT, S], bf)

        # ---------------- pass 1: stream x, q conv + pooling ----------------
        with tc.tile_pool(name="p1ps", bufs=2, space="PSUM") as p1ps, \
             tc.tile_pool(name="poolps", bufs=1, space="PSUM") as poolps, \
             tc.tile_pool(name="xtp", bufs=2) as xtp:
            pooledT_ps = poolps.tile([110, C], mybir.dt.float32)
            for ci in range(NBIG):
                nsl = slice(ci * CHUNK, (ci + 1) * CHUNK)
                nc.sync.dma_start(out=x_sb[:, :, nsl], in_=xv[:, :, nsl])
                xt_t = xtp.tile([128, CHUNK // 128, C], bf, tag="xt")
                nc.gpsimd.dma_start(
                    out=xt_t, in_=xtv[:, ci * (CHUNK // 128):(ci + 1) * (CHUNK // 128), :])
                for sub in range(CHUNK // SUB):
                    ns2 = slice(ci * CHUNK + sub * SUB, ci * CHUNK + (sub + 1) * SUB)
                    q_ps = p1ps.tile([128, MT_O, SUB], mybir.dt.float32, tag="qps")
                    for m in range(MT_O):
                        for k in range(KT):
                            nc.tensor.matmul(
                                q_ps[:, m, :], wq_sb[:, k, m * 128:(m + 1) * 128],
                                x_sb[:, k, ns2], start=(k == 0), stop=(k == KT - 1),
                                skip_group_check=True)
                        nc.scalar.activation(
                            q_sb[:, m, ns2], q_ps[:, m, :], AF.Identity,
                            bias=bq_sb[:, m, :], scale=1.0)
                for kt in range(CHUNK // 128):
                    kg = ci * (CHUNK // 128) + kt
                    nc.tensor.matmul(
                        pooledT_ps, mp_sb[:, kg, :], xt_t[:, kt, :],
                        start=(kg == 0), stop=(kg == NT - 1),
                        skip_group_check=True)

            # ---------------- finalize pools ----------------
            nc.scalar.copy(pooledT_sb, pooledT_ps)
            for c in range(KT):
                tr_ps = p1ps.tile([128, S], mybir.dt.bfloat16, tag="trps")
                nc.tensor.transpose(
                    tr_ps, pooledT_sb[:, c * 128:(c + 1) * 128],
                    id_sb[:110, :110])
                nc.scalar.copy(pooled_sb[:, c, :], tr_ps)
            kp_ps = p1ps.tile([128, MT_O, 128], mybir.dt.float32, tag="trps")
            for m in range(MT_O):
                for k in range(KT):
                    nc.tensor.matmul(
                        kp_ps[:, m, 0:S], wk_sb[:, k, m * 128:(m + 1) * 128],
                        pooled_sb[:, k, :], start=(k == 0), stop=False,
                        skip_group_check=True)
                nc.tensor.matmul(
                    kp_ps[:, m, 0:S], bk_sb[0:1, m * 128:(m + 1) * 128],
                    ones_sb[0:1, 0:S], start=False, stop=True,
                    skip_group_check=True)
            nc.scalar.copy(k_pool_sb, kp_ps[:, :, 0:S])
            vp_ps = p1ps.tile([110, C], mybir.dt.float32, tag="qps")
            for k in range(KT):
                nc.tensor.matmul(vp_ps, pooled_sb[:, k, :], wv_sb[:, k, :],
                                 start=(k == 0), stop=False,
                                 skip_group_check=True)
            nc.tensor.matmul(vp_ps, ones_sb[0:1, :110], bv_sb,
                             start=False, stop=True, skip_group_check=True)
            nc.scalar.copy(vT_sb, vp_ps)

        # ---------------- pass 2: attention + output ----------------
        with tc.tile_pool(name="p2ps", bufs=2, space="PSUM") as p2ps, \
             tc.tile_pool(name="pops", bufs=3, space="PSUM") as pops, \
             tc.tile_pool(name="p2sb", bufs=3) as p2sb, \
             tc.tile_pool(name="outp", bufs=3) as outp:
            for ci in range(N // SUB):
                ns2 = slice(ci * SUB, (ci + 1) * SUB)
                at_ps = p2ps.tile([110, SUB], mybir.dt.float32, tag="atdb")
                for kt in range(MT_O):
                    nc.tensor.matmul(at_ps, k_pool_sb[:, kt, :],
                                     q_sb[:, kt, ns2],
                                     start=(kt == 0), stop=(kt == MT_O - 1))
                exp_sb = p2sb.tile([110, SUB], mybir.dt.bfloat16, tag="exp")
                nc.scalar.activation(exp_sb, at_ps, AF.Exp)
                db_ps = p2ps.tile([110, SUB], mybir.dt.float32, tag="atdb")
                nc.tensor.matmul(db_ps, ones_sb[:110, :110], exp_sb,
                                 start=True, stop=True)
                recip_sb = p2sb.tile([110, SUB], mybir.dt.float32, tag="recip")
                nc.vector.reciprocal_approx_fast(recip_sb, db_ps)
                attn_sb = p2sb.tile([110, SUB], mybir.dt.bfloat16, tag="attn")
                with nc.allow_low_precision("softmax weights tolerate bf16"):
                    nc.gpsimd.tensor_mul(attn_sb, exp_sb, recip_sb)
                out_t = outp.tile([128, KT, SUB], mybir.dt.float32, tag="out")
                # c-tiles 0,1: residual add fused into the psum->sbuf move (DVE)
                o_psa = pops.tile([128, 2, SUB], mybir.dt.float32, tag="ops")
                for c in range(2):
                    nc.tensor.matmul(o_psa[:, c, :],
                                     vT_sb[:, c * 128:(c + 1) * 128],
                                     attn_sb, start=True, stop=True,
                                     skip_group_check=True)
                with nc.allow_low_precision("fp32 psum + bf16 residual"):
                    nc.vector.tensor_add(out_t[:, 0:2, :], o_psa,
                                         x_sb[:, 0:2, ns2])
                # c-tiles 2,3: residual via identity matmul (PE), copy on ACT
                o_psb = pops.tile([128, 2, SUB], mybir.dt.float32, tag="ops")
                for c in range(2, KT):
                    nc.tensor.matmul(o_psb[:, c - 2, :],
                                     vT_sb[:, c * 128:(c + 1) * 128],
                                     attn_sb, start=True, stop=False,
                                     skip_group_check=True)
                    nc.tensor.matmul(o_psb[:, c - 2, :], id_sb,
                                     x_sb[:, c, ns2],
                                     start=False, stop=True,
                                     skip_group_check=True)
                nc.scalar.copy(out_t[:, 2:4, :], o_psb)
                nc.gpsimd.dma_start(out=outv[:, :, ns2], in_=out_t)

    nc.compile()
    return nc


_NC_CACHE = None


def _get_nc() -> bacc.Bacc:
    global _NC_CACHE
    if _NC_CACHE is None:
        _NC_CACHE = build_nc()
    return _NC_CACHE


def _prep_in_maps(x, Wq, bq, Wk, bk, Wv, bv):
    shared = {
        "mpool": _build_pool_matrix().astype(BF16),
        "wqT": np.ascontiguousarray(Wq.T).astype(BF16),
        "wkT": np.ascontiguousarray(Wk.T).astype(BF16),
        "wvT": np.ascontiguousarray(Wv.T).astype(BF16),
        "bq": np.ascontiguousarray(bq.reshape(O, 1)).astype(np.float32),
        "bk_bf": np.ascontiguousarray(bk.reshape(1, O)).astype(BF16),
        "bv_bf": np.ascontiguousarray(bv.reshape(1, C)).astype(BF16),
        "ones_bf": np.ones((128, 512), dtype=BF16),
        "ident_bf": np.eye(128, dtype=np.float32).astype(BF16),
    }
    in_maps = []
    for i in range(NCORES):
        xi = np.ascontiguousarray(x[i].reshape(C, N))
        m = dict(shared)
        m["x_bf"] = xi.astype(BF16)
        m["xT_bf"] = np.ascontiguousarray(xi.T).astype(BF16)
        in_maps.append(m)
    return in_maps


def _install_ntff_hook():
    """The agent image ships no antenv.axon_hooks module, so trace=True
    under axon crashes on import. Recreate the tiny hook-holder module and
    register trn_boot's ctypes NTFF hook so neuron-profile timing works."""
    import sys
    import types
    if "antenv.axon_hooks" in sys.modules:
        return
    mod = types.ModuleType("antenv.axon_hooks")
    holder = {"h": None}
    mod.set_axon_ntff_profile_hook = lambda h: holder.__setitem__("h", h)
    mod.get_axon_ntff_profile_hook = lambda: holder["h"]
    sys.modules["antenv.axon_hooks"] = mod
    try:
        import antenv
        antenv.axon_hooks = mod
    except ImportError:
        pass
    try:
        from trn_agent_boot.trn_boot import _ntff_profile_via_ctypes
        mod.set_axon_ntff_profile_hook(
            _ntff_profile_via_ctypes("/opt/axon/libaxon_pjrt.so"))
    except Exception as e:  # degrade to no profiling
        print(f"ntff hook install failed: {e}")


def _run(trace: bool, **inputs):
    if trace:
        _install_ntff_hook()
        import concourse.bass_utils as bu
        bu.upload_artifacts = lambda tmpdir: tmpdir  # no cloud bucket here
    nc = _get_nc()
    in_maps = _prep_in_maps(
        inputs["x"], inputs["Wq"], inputs["bq"], inputs["Wk"], inputs["bk"],
        inputs["Wv"], inputs["bv"])
    res = run_bass_kernel_spmd(nc, in_maps, core_ids=list(range(NCORES)),
                               trace=trace)
    out = np.stack([
        np.asarray(res.results[i]["out"]).reshape(C, H, W)
        for i in range(NCORES)
    ]).astype(np.float32)
    return out, res


def kernel(**inputs) -> np.ndarray:
    out, _ = _run(False, **inputs)
    return out


def kernel_profiled(**inputs):
    out, res = _run(True, **inputs)
    return out, res
